# revision 5
# baseline (speedup 1.0000x reference)
"""GCNConv Trainium2 Bass kernel v3 (8 NeuronCores, SPMD, AllToAll).

out = (1/DEG) * A X W + b for the deterministic degree-regular circulant
graph (node i -> (i + off_j) % N), offsets baked at compile time.

Design (fully static SPMD; core c ends up owning output COLUMNS
[8c, 8c+8) for ALL N nodes):
  - phase A: hT = (x_shard @ W).T computed directly transposed:
    psum[d, rows] = w_chunk.T @ xt_chunk, drained via Activation to fp16
    and stored as hT [64, SH_A] (feature-major) in DRAM.
  - AllToAll: input block c = hT[8c:8c+8, :] (my rows, columns 8c..8c+8);
    output block i = core i's columns [8c, 8c+8) -> hcolT [8][8, SH_A]
    = my 8 columns for every node.  Only 1.6MB through the collective.
  - hfT SBUF tile [128, 6250] fp16: partition p = grp*8 + d' holds
    h[(grp*6250 + v'), 8c+d'] (16 row-groups; N = 16*6250 exactly, so
    the mod-N ring wrap is a clean rotation of the 16 groups).
    Loaded by ~23 static DMAs (group extents split at 12544 shard seams).
  - phase W: out[v] = sum_j h[(v + off_j) % N].  off_j = qg*6250 + dv.
    Per window two v'-branches (v' < 6250-dv: group shift qg; else
    qg+1 with v'' = v'+dv-6250), split by v'-ranges across engines:
      PE:   one matmul per 512-chunk with lhsT = full rotation R_qg
            (mod-128 partition rotation is exactly the mod-N wrap)
      DVE:  2 partition-window fp16 adds per branch
      Pool: same on gpsimd
    Bias: PE seeds psum via bvec_row x ones; DVE/Pool seed via
    tensor_scalar add of bvec_col on the first window.
  - single fp16 output out_all [128 * 6250]; host reassembles columns.
"""

from contextlib import ExitStack

import numpy as np

import concourse.bacc as bacc
import concourse.bass as bass
import concourse.mybir as mybir
import concourse.tile as tile

NG = 16  # row groups


def plan_v3(N, DIN, D, DEG, M, offsets):
    offsets = sorted(int(o) for o in offsets)
    DC = D // M                  # 8 output columns per core
    SH_A = 12544                 # aligned shard rows; core M-1 ragged
    assert M * SH_A >= N and (M - 1) * SH_A < N
    assert N % NG == 0
    VG = N // NG                 # 6250
    assert DC * NG == 128
    # PE covers all of [0, VG) in psum passes of <= PWB (6 banks; psumA
    # keeps the other 2 for phase-A overlap across reps)
    PWB = 3072
    passes = [(a, min(a + PWB, VG)) for a in range(0, VG, PWB)]
    # hfT load pieces: for each group, [grp*VG, (grp+1)*VG) split at
    # shard seams (12544*i): list of (grp, v0, len, src_shard, src_r0)
    pieces = []
    for g in range(NG):
        v0 = g * VG
        v1 = v0 + VG
        cuts = [v0] + [i * SH_A for i in range(1, M)
                       if v0 < i * SH_A < v1] + [v1]
        for a, b in zip(cuts[:-1], cuts[1:]):
            i = a // SH_A
            pieces.append((g, a - v0, b - a, i, a - i * SH_A))
    return dict(offsets=offsets, DC=DC, SH_A=SH_A, VG=VG,
                PWB=PWB, passes=passes, pieces=pieces)


def build_gcn_nc_v3(
    N: int, DIN: int, D: int, DEG: int, M: int,
    offsets,
    reps: int = 1,
    phases: str = "ACLW",   # A=matmul, C=alltoall, L=hfT load, W=windows
):
    P = plan_v3(N, DIN, D, DEG, M, offsets)
    offs = P["offsets"]
    DC, SH_A, VG = P["DC"], P["SH_A"], P["VG"]
    PWB, passes = P["PWB"], P["passes"]
    pieces = P["pieces"]
    KC = (DIN + 127) // 128
    assert DIN % KC == 0
    KSZ = DIN // KC

    f32 = mybir.dt.float32
    f16 = mybir.dt.float16

    nc = bacc.Bacc("TRN2", num_devices=M)

    xt = nc.dram_tensor("xt", [DIN, SH_A], f16, kind="ExternalInput")
    w = nc.dram_tensor("w", [DIN, D], f16, kind="ExternalInput")
    rots = nc.dram_tensor("rots", [128, NG * 128], f16, kind="ExternalInput")
    bvec_col = nc.dram_tensor("bvec_col", [128, 1], f32, kind="ExternalInput")
    bvec_row = nc.dram_tensor("bvec_row", [1, 128], f16, kind="ExternalInput")
    ones512 = nc.dram_tensor("ones512", [1, 512], f16, kind="ExternalInput")
    out_all = nc.dram_tensor("out_all", [reps * 128 * VG], f16,
                             kind="ExternalOutput")

    hT = nc.dram_tensor("hT", [D * SH_A], f16)
    hcolT = nc.dram_tensor("hcolT", [M * DC * SH_A], f16)

    qs = [o // VG for o in offs]     # group shifts
    dvs = [o % VG for o in offs]     # v' shifts

    # phase A tiling: rows processed in psum chunks of 512
    ACH = 512
    NACH = SH_A // ACH               # 24.5 -> must divide; 12544/512=24.5!
    # 12544 = 24*512 + 256: use 24 chunks of 512 + 1 of 256
    a_chunks = [(i * ACH, min((i + 1) * ACH, SH_A)) for i in range(-(-SH_A // ACH))]

    with tile.TileContext(nc) as tc, ExitStack() as ctx:
        pconst = ctx.enter_context(tc.tile_pool(name="pconst", bufs=1))
        pxt = ctx.enter_context(tc.tile_pool(name="pxt", bufs=2))
        pht = ctx.enter_context(tc.tile_pool(name="pht", bufs=3))
        phfT = ctx.enter_context(tc.tile_pool(name="phfT", bufs=2))
        psumA = ctx.enter_context(tc.tile_pool(name="psumA", bufs=2, space="PSUM"))
        psumW = ctx.enter_context(tc.tile_pool(name="psumW", bufs=1, space="PSUM"))
        pacc = ctx.enter_context(tc.tile_pool(name="pacc", bufs=1))
        for _rep in range(reps):
            # ---------------- phase A: hT = (xt.T @ w).T ----------------
            w_sb = pconst.tile([KSZ, KC * D], f16, tag="w")
            w_r = w.rearrange("(c p) d -> p c d", c=KC, p=KSZ)
            nc.sync.dma_start(out=w_sb[:], in_=w_r[:, :, :])

            rots_sb = pconst.tile([128, NG * 128], f16, tag="rots")
            nc.scalar.dma_start(out=rots_sb[:], in_=rots[:, :])
            bcol_sb = pconst.tile([128, 1], f32, tag="bcol")
            nc.scalar.dma_start(out=bcol_sb[:], in_=bvec_col[:, :])
            brow_sb = pconst.tile([1, 128], f16, tag="brow")
            nc.scalar.dma_start(out=brow_sb[:], in_=bvec_row[:, :])
            ones_sb = pconst.tile([1, 512], f16, tag="ones")
            nc.scalar.dma_start(out=ones_sb[:], in_=ones512[:, :])

            xt_r = xt.rearrange("(c p) s -> p c s", c=KC, p=KSZ)
            XCH = 2048   # xt chunk rows (4 psum chunks; last chunk ragged)

            if "A" in phases:
                # chunk pairs share one [128, ACH] psum tile via PE column
                # tiling: even chunk -> col-group 0 (partitions 0:64), odd
                # chunk -> col-group 64 (partitions 64:128), concurrent on PE
                xt_ch, pA2 = None, None
                for ci, (r0, r1) in enumerate(a_chunks):
                    if r0 % XCH == 0:
                        m = r0 // XCH
                        ext = min(XCH, SH_A - m * XCH)
                        xt_ch = pxt.tile([KSZ, KC * XCH], f16, tag="xtc",
                                         name=f"xtc{_rep}_{m}")
                        for c in range(KC):
                            nc.sync.dma_start(
                                out=xt_ch[:, c * XCH:c * XCH + ext],
                                in_=xt_r[:, c:c + 1,
                                         m * XCH:m * XCH + ext])
                    rl = r1 - r0
                    lo = r0 % XCH
                    half = ci % 2
                    if half == 0:
                        pA2 = psumA.tile([128, ACH], f32, tag="pA",
                                         name=f"pA{_rep}_{ci // 2}")
                    for c in range(KC):
                        nc.tensor.matmul(
                            out=pA2[64 * half:64 * half + 64, 0:rl],
                            lhsT=w_sb[:, c * D:(c + 1) * D],
                            rhs=xt_ch[:, c * XCH + lo:c * XCH + lo + rl],
                            start=(c == 0), stop=(c == KC - 1),
                            tile_position=(0, 64 * half),
                        )
                    if half == 1 or ci == len(a_chunks) - 1:
                        np_ = 64 * (half + 1)
                        hT_sb = pht.tile([128, ACH], f16, tag="hts",
                                         name=f"hts{_rep}_{ci // 2}")
                        nc.scalar.copy(out=hT_sb[0:np_, 0:rl],
                                       in_=pA2[0:np_, 0:rl])
                        if half == 0:   # unpaired tail chunk
                            nc.sync.dma_start(
                                out=bass.AP(hT, r0, [[SH_A, D], [1, rl]]),
                                in_=hT_sb[0:64, 0:rl])
                        else:
                            nc.sync.dma_start(
                                out=bass.AP(hT, r0 - ACH,
                                            [[SH_A, D], [1, ACH]]),
                                in_=hT_sb[0:64, 0:ACH])
                            nc.sync.dma_start(
                                out=bass.AP(hT, r0, [[SH_A, D], [1, rl]]),
                                in_=hT_sb[64:128, 0:rl])

            # ---------------- AllToAll ----------------
            if "C" in phases:
                nc.gpsimd.collective_compute(
                    "AllToAll",
                    mybir.AluOpType.bypass,
                    replica_groups=[list(range(M))],
                    ins=[hT.ap().opt()],
                    outs=[hcolT.ap().opt()],
                )

            if "L" not in phases:
                continue

            # ---------------- hfT load ----------------
            # hfT[p = grp*8 + d', v'] = h[grp*VG + v', 8c + d']
            hfT = phfT.tile([128, VG], f16, tag="hfT", name=f"hfT{_rep}")
            lengs = [nc.sync, nc.scalar, nc.gpsimd]
            for pi, (g, voff, ln, i, r0) in enumerate(pieces):
                eng = lengs[pi % len(lengs)]
                eng.dma_start(
                    out=hfT[g * DC:(g + 1) * DC, voff:voff + ln],
                    in_=bass.AP(hcolT, i * DC * SH_A + r0,
                                [[SH_A, DC], [1, ln]]),
                )

            if "W" not in phases:
                continue

            # ---------------- phase W (PE rotation passes) ----------------
            # branches per window: (v'-range, rotation, src free offset)
            def branches(jj):
                q, dv = qs[jj], dvs[jj]
                if dv == 0:
                    return [(0, VG, q % NG, 0)]
                return [(0, VG - dv, q % NG, dv),
                        (VG - dv, VG, (q + 1) % NG, dv - VG)]

            # Offload the heaviest rotation classes to DVE: per-class fp16
            # accumulators kept in the ROTATED frame (adds stay fully
            # partition-aligned -- engines cannot shift partitions at all);
            # PE merges each class with one rotation matmul per pass chunk.
            # A class pays ~VG of PE merge, so offload only classes whose
            # branch units exceed VG, largest first, DVE-budget-capped.
            cls_units = {}
            for jj in range(DEG):
                for (b0, b1, rot, foff) in branches(jj):
                    cls_units[rot] = cls_units.get(rot, 0) + (b1 - b0)
            cand = sorted((u, r) for r, u in cls_units.items()
                          if u > VG * 1.15)
            cand.reverse()
            offl_classes, dve_units = [], 0
            for u, r in cand:
                if dve_units + u > 50000:   # ~26us DVE cap
                    continue
                offl_classes.append(r)
                dve_units += u
            offl_set = set(offl_classes)

            accs = {}
            for r in offl_classes:
                accs[r] = pacc.tile([128, VG], f16, tag=f"accr{r}",
                                    name=f"accr{_rep}_{r}")
            # first-touch interval bookkeeping: copy on new cols, add on
            # covered cols; memset any never-covered remainder
            covered = {r: [] for r in offl_classes}

            def _touch(r, lo, hi):
                """Split [lo,hi) into (copy, add) pieces vs covered[r]."""
                segs = []
                cur = lo
                for (a, b) in sorted(covered[r]):
                    if b <= lo or a >= hi:
                        continue
                    if cur < a:
                        segs.append((cur, min(a, hi), "copy"))
                    segs.append((max(cur, a), min(b, hi), "add"))
                    cur = min(b, hi)
                if cur < hi:
                    segs.append((cur, hi, "copy"))
                covered[r].append((lo, hi))
                # normalize covered to merged intervals
                merged = []
                for (a, b) in sorted(covered[r]):
                    if merged and a <= merged[-1][1]:
                        merged[-1] = (merged[-1][0], max(b, merged[-1][1]))
                    else:
                        merged.append((a, b))
                covered[r] = merged
                return [s for s in segs if s[0] < s[1]]

            for jj in range(DEG):
                for (b0, b1, rot, foff) in branches(jj):
                    if rot not in offl_set:
                        continue
                    for (s0, s1, kind) in _touch(rot, b0, b1):
                        if kind == "copy":
                            nc.vector.tensor_copy(
                                out=accs[rot][:, s0:s1],
                                in_=hfT[:, s0 + foff:s1 + foff])
                        else:
                            nc.vector.tensor_add(
                                out=accs[rot][:, s0:s1],
                                in0=accs[rot][:, s0:s1],
                                in1=hfT[:, s0 + foff:s1 + foff])
            # zero never-covered gaps so pass merges can read uniformly
            for r in offl_classes:
                cur = 0
                for (a, b) in covered[r] + [(VG, VG)]:
                    if cur < a:
                        nc.gpsimd.memset(accs[r][:, cur:a], 0)
                    cur = max(cur, b)
            any_offl = bool(offl_classes)

            ptW = psumW.tile([128, PWB], f32, tag="ptW")
            for pi_, (P0, P1) in enumerate(passes):
                # chunk grid: 512-bank marks + all branch boundaries
                cuts = set(range(P0, P1, 512)) | {P1}
                cuts |= {VG - dv for dv in dvs if P0 < VG - dv < P1}
                cuts = sorted(cuts)
                pw_chunks = list(zip(cuts[:-1], cuts[1:]))
                # bias seed: ptW[p, v'] = bvec_row[0, p] * 1
                for (c0, c1) in pw_chunks:
                    nc.tensor.matmul(
                        out=ptW[:, c0 - P0:c1 - P0], lhsT=brow_sb[:, :],
                        rhs=ones_sb[0:1, 0:c1 - c0], start=True, stop=False,
                    )
                # last PE toucher per chunk gets stop=True
                emit = []
                for jj in range(DEG):
                    brs = branches(jj)
                    for ki, (c0, c1) in enumerate(pw_chunks):
                        (b0, b1, qg, foff) = next(
                            b for b in brs if b[0] <= c0 and c1 <= b[1])
                        if qg in offl_set:
                            continue
                        emit.append([ki, c0, c1, qg, hfT, foff])
                for r in offl_classes:
                    for ki, (c0, c1) in enumerate(pw_chunks):
                        emit.append([ki, c0, c1, r, accs[r], 0])
                last_of = {}
                for mi, e in enumerate(emit):
                    last_of[e[0]] = mi
                stops = set(last_of.values())
                for mi, (ki, c0, c1, qg, src, foff) in enumerate(emit):
                    nc.tensor.matmul(
                        out=ptW[:, c0 - P0:c1 - P0],
                        lhsT=rots_sb[:, qg * 128:(qg + 1) * 128],
                        rhs=src[:, c0 + foff:c1 + foff],
                        start=False, stop=mi in stops,
                    )
                # drain this pass: psum -> fp16 -> DRAM
                pe_out = pacc.tile([128, P1 - P0], f16, tag=f"peout{pi_}",
                                   name=f"peout{_rep}_{pi_}")
                nc.scalar.copy(out=pe_out[:], in_=ptW[:, 0:P1 - P0])
                nc.sync.dma_start(
                    out=bass.AP(out_all, _rep * 128 * VG + P0,
                                [[VG, 128], [1, P1 - P0]]),
                    in_=pe_out[:])

    nc.compile()
    return nc, P


def make_inputs_v3(N, DIN, D, DEG, M, x, weight, bias, offsets, scale, P):
    offs = P["offsets"]
    DC, SH_A, VG = P["DC"], P["SH_A"], P["VG"]
    xt_full = np.ascontiguousarray(x.T).astype(np.float16)  # [DIN, N]
    w_eff = (weight.astype(np.float32) * np.float32(scale)).astype(np.float16)
    rots = np.zeros((128, NG * 128), np.float16)
    qset = sorted({0} | set((o // VG) % NG for o in offs) |
                  set((o // VG + 1) % NG for o in offs))
    for qg in qset:
        shift = (qg * DC) % 128
        for p in range(128):
            rots[(p + shift) % 128, qg * 128 + p] = 1.0
    ones512 = np.ones((1, 512), np.float16)
    in_maps = []
    for k in range(M):
        base = k * SH_A
        valid = min(SH_A, N - base)
        xt_k = np.zeros((DIN, SH_A), np.float16)
        xt_k[:, :valid] = xt_full[:, base:base + valid]
        bcol = bias.astype(np.float32)[DC * k + (np.arange(128) % DC)][:, None]
        in_maps.append({
            "xt": xt_k,
            "w": w_eff,
            "rots": rots,
            "bvec_col": bcol,
            "bvec_row": bcol.T.astype(np.float16).copy(),
            "ones512": ones512,
        })
    return in_maps


def assemble_output_v3(res, N, D, M, P):
    DC, VG = P["DC"], P["VG"]
    full = np.empty((N, D), np.float32)
    for k in range(M):
        arr = np.asarray(res.results[k]["out_all"])[:128 * VG].astype(np.float32)
        arr = arr.reshape(NG, DC, VG)           # (grp, d', v')
        full[:, DC * k:DC * (k + 1)] = arr.transpose(0, 2, 1).reshape(N, DC)
    return full


# ---------------------------------------------------------------------------
# Host-side entry point
# ---------------------------------------------------------------------------

_CACHE = {}


def _get_nc(N, DIN, D, DEG, M, offsets):
    key = (N, DIN, D, DEG, M, tuple(int(o) for o in offsets))
    if key not in _CACHE:
        _CACHE.clear()   # one compiled program at a time (keeps memory sane)
        _CACHE[key] = build_gcn_nc_v3(N, DIN, D, DEG, M, offsets)
    return _CACHE[key]


def _is_circulant(N, DEG, rowptr, colind, colptr):
    if rowptr.shape[0] != N + 1 or colind.shape[0] != N * DEG:
        return None
    if not np.array_equal(rowptr.astype(np.int64),
                          np.arange(N + 1, dtype=np.int64) * DEG):
        return None
    if not np.array_equal(colptr, rowptr):
        return None
    offsets = colind[:DEG].astype(np.int64)
    if offsets.min() < 1 or offsets.max() >= N or len(set(offsets.tolist())) != DEG:
        return None
    rows = np.arange(N, dtype=np.int64)
    expect = np.sort((rows[:, None] + offsets[None, :]) % N, axis=1).reshape(-1)
    if not np.array_equal(colind.astype(np.int64), expect):
        return None
    return np.sort(offsets)


def _kernel_numpy_fallback(x, weight, bias, rowptr, colind, colptr):
    # general-graph fallback (never taken for the deterministic circulant
    # inputs this problem generates; correctness insurance only)
    h = x.astype(np.float32) @ weight.astype(np.float32)
    out_deg = (colptr[1:] - colptr[:-1]).astype(np.float32)
    in_deg = (rowptr[1:] - rowptr[:-1]).astype(np.float32)
    h = h * (1.0 / np.sqrt(np.maximum(out_deg, 1e-30)))[:, None]
    N = rowptr.shape[0] - 1
    E = colind.shape[0]
    row_ids = np.searchsorted(rowptr, np.arange(E), side="right") - 1
    aggr = np.zeros_like(h)
    np.add.at(aggr, row_ids, h[colind])
    aggr = aggr * (1.0 / np.sqrt(np.maximum(in_deg, 1e-30)))[:, None]
    return (aggr + bias).astype(np.float32)


def _geometry_ok(N, DIN, D, DEG, M):
    try:
        plan_v3(N, D and DIN, D, DEG, M, [1] * DEG)
    except Exception:
        return False
    return (N % NG == 0 and (D % M) == 0 and (D // M) * NG == 128
            and DIN % 128 == 0)


def kernel(x, weight, bias, rowptr, colind, colptr, rowind=None, **_unused):
    """GCNConv: out = D_in^-1/2 A D_out^-1/2 X W + b, distributed over 8
    NeuronCores: nodes sharded for the feature matmul, AllToAll into
    column shards, fully local circulant aggregation (offsets baked)."""
    from concourse.bass_utils import run_bass_kernel_spmd

    x = np.asarray(x)
    weight = np.asarray(weight)
    bias = np.asarray(bias)
    rowptr = np.asarray(rowptr)
    colind = np.asarray(colind)
    colptr = np.asarray(colptr)

    N, DIN = x.shape
    D = weight.shape[1]
    M = 8
    DEG = colind.shape[0] // max(N, 1)

    offsets = _is_circulant(N, DEG, rowptr, colind, colptr)
    if offsets is None or not _geometry_ok(N, DIN, D, DEG, M):
        return _kernel_numpy_fallback(x, weight, bias, rowptr, colind, colptr)

    # degree-regular graph: both rsqrt scalings are exactly 1/DEG, folded
    # into W on the host
    scale = 1.0 / DEG

    nc, P = _get_nc(N, DIN, D, DEG, M, offsets)
    in_maps = make_inputs_v3(N, DIN, D, DEG, M, x, weight, bias, offsets,
                             scale, P)
    res = run_bass_kernel_spmd(nc, in_maps, list(range(M)))
    return assemble_output_v3(res, N, D, M, P).astype(np.float32)
